# revision 30
# baseline (speedup 1.0000x reference)
"""Trainium2 Bass kernel for 16-head self-attention (N=4, S=2048, E=1024).

Sharding: 8 cores = 4 batches x 2 head-groups (8 heads each).  Each core
computes its head-group's attention and a partial fc_out product
(S x 1024); the host sums the two partials per batch and adds the bias.

v2 structure (vs the v1 baseline):
  - Q/K projections fused into a single P = G @ Xk pass (G = Wq.T Wk,
    host-precomputed); scores = P.T @ Xq uses the raw query input.
  - V projection folded into fc_out: ctx uses raw Xv (host-marshaled
    with a fused ones column for sumexp), fc weights M_h = Wv.T Wo_h.T.
  - exp split between ScalarE (LUT exp) and DVE (Schraudolph bit-trick
    exp: i16 = round(x*a+b) bitcast bf16) so the Activation engine is no
    longer the serial wall.
  - fc_out software-pipelined one query-chunk behind attention so PE
    never stalls on the normalization chain; ACT does the fc PSUM->SBUF
    copies.
"""

import numpy as np
import ml_dtypes

EMBED = 1024
HEADS = 16
HD = 64
N_CORES = 8
HPC = 8  # heads per core
GCOLS = HPC * HD  # 512 embed columns per core

# jb indices (of 16 key-blocks) whose exp runs on DVE via Schraudolph.
DVE_JBS = (2, 6, 10, 13)
SCHRAUDOLPH_C = 0.0435


def ts(i, n):
    return slice(i * n, (i + 1) * n)


def build_program(S, dve_jbs=DVE_JBS, taps=False):
    import concourse.bass as bass
    import concourse.tile as tile
    import concourse.mybir as mybir
    from concourse import bacc

    f32 = mybir.dt.float32
    f32r = mybir.dt.float32r
    bf16 = mybir.dt.bfloat16
    i16 = mybir.dt.int16
    f8 = mybir.dt.float8e4
    EXP = mybir.ActivationFunctionType.Exp
    MUL = mybir.AluOpType.mult
    ADD = mybir.AluOpType.add
    DROW = mybir.MatmulPerfMode.DoubleRow

    NPAIR = 4
    NJB = S // 128  # 16 key blocks
    NJB2 = NJB // 2
    NCC = S // 512  # query chunks

    inv_sqrt_e = 1.0 / float(np.sqrt(EMBED))
    # Schraudolph constants: i16 = round(score * A16 + B16), bitcast bf16
    A16 = 128.0 * np.log2(np.e) * inv_sqrt_e
    B16 = 127.0 * 128.0 - SCHRAUDOLPH_C * 128.0

    nc = bacc.Bacc("TRN2", target_bir_lowering=False, debug=False)

    xk_d = nc.dram_tensor("xk", [NPAIR, 128, S], bf16, kind="ExternalInput").ap()
    # queries pre-interleaved for DoubleRow fp8: [pr][p, (h, i2, n)],
    # embed dim d = h*64 + i2*32 + p
    xq_d = nc.dram_tensor("xq", [NPAIR, 32, 4 * S], f8, kind="ExternalInput").ap()
    vp_d = nc.dram_tensor("vp", [NJB2, 128, NPAIR * 2 * 2 * 65], bf16, kind="ExternalInput").ap()
    gs_d = nc.dram_tensor("gs", [128, HD], bf16, kind="ExternalInput").ap()
    ms_d = nc.dram_tensor("ms", [NPAIR, 128, EMBED], f32r, kind="ExternalInput").ap()
    out_d = nc.dram_tensor("out", [S, EMBED], f32, kind="ExternalOutput").ap()
    rec_dram = nc.dram_tensor("rec_scratch", [NPAIR, 2, S], f32)
    tap_d = {}
    if taps:
        for nm, shape, dt_ in (
            ("dbg_P", [128, S], bf16),
            ("dbg_e2", [128, 2048], bf16),
            ("dbg_cu", [65, 512], f32),
            ("dbg_ctxT", [128, S], f32),
            ("dbg_ctxT1", [128, S], f32),
            ("dbg_ctxT2", [128, S], f32),
            ("dbg_ctxT3", [128, S], f32),
        ):
            tap_d[nm] = nc.dram_tensor(nm, shape, dt_, kind="ExternalOutput").ap()

    with tile.TileContext(nc) as tc:
        import contextlib

        with contextlib.ExitStack() as ctx:
            # ---- persistent pools ----
            const_p = ctx.enter_context(tc.tile_pool(name="const", bufs=1))
            pq_p = ctx.enter_context(tc.tile_pool(name="pq", bufs=1))
            vp_p = ctx.enter_context(tc.tile_pool(name="vp", bufs=1))
            ctxT_p = ctx.enter_context(tc.tile_pool(name="ctxT", bufs=1))
            wo_p = ctx.enter_context(tc.tile_pool(name="wo", bufs=1))
            # PSUM: sc slots 2 banks x2 bufs + ctx slots 2 banks x2 bufs = 8
            sc_ps = ctx.enter_context(tc.tile_pool(name="sc", bufs=2, space="PSUM"))
            ctx_ps = ctx.enter_context(tc.tile_pool(name="ctxps", bufs=2, space="PSUM"))

            gs_s = const_p.tile([128, HD], bf16, tag="gs")
            nc.sync.dma_start(gs_s[:], gs_d[:])
            ms_t = [wo_p.tile([128, EMBED], f32r, tag=f"ms{p}", name=f"ms{p}") for p in range(NPAIR)]

            P_s = [pq_p.tile([128, S], bf16, tag=f"P{p}", name=f"P{p}") for p in range(NPAIR)]
            # fp8 DoubleRow operand tiles: [32, (h, i2, S)]
            Pp_s = [pq_p.tile([32, 4 * S], f8, tag=f"Pp{p}", name=f"Pp{p}") for p in range(NPAIR)]
            xq_s = [pq_p.tile([32, 4 * S], f8, tag=f"xq{p}", name=f"xq{p}") for p in range(NPAIR)]
            vp_t = [vp_p.tile([128, NPAIR * 260], bf16, tag=f"vp{j}", name=f"vp{j}") for j in range(NJB2)]
            ctxT = [ctxT_p.tile([128, S], f32r, tag=f"cx{p}", name=f"cx{p}") for p in range(NPAIR)]

            xk_t = [pq_p.tile([128, S], bf16, tag=f"xk{p}", name=f"xk{p}") for p in range(NPAIR)]
            for p in range(NPAIR):
                nc.sync.dma_start(xk_t[p][:], xk_d[p])
            nc.sync.dma_start(xq_s[0][:], xq_d[0])
            for j in range(NJB2):
                nc.sync.dma_start(vp_t[j][:], vp_d[j])
            for p in range(1, NPAIR):
                nc.sync.dma_start(xq_s[p][:], xq_d[p])
            for p in range(NPAIR):
                nc.sync.dma_start(ms_t[p][:], ms_d[p])

            # ---- P = G.T-stacked @ xk (key-side fused projection) ----
            # P(0) upfront; P(1..3) ride inside the cc=0 blocks.
            def p_proj(pr):
                for chh in range(S // 1024):
                    ps = sc_ps.tile([128, 1024], f32, tag="sc")
                    for ch2 in range(2):
                        for b in (0, 64):
                            nc.tensor.matmul(
                                ps[b : b + 64, ts(ch2, 512)],
                                lhsT=gs_s[b : b + 64, :],
                                rhs=xk_t[pr][b : b + 64, ts(chh * 2 + ch2, 512)],
                                start=True,
                                stop=True,
                            )
                    if chh % 2 == 0:
                        nc.scalar.copy(P_s[pr][:, ts(chh, 1024)], ps[:])
                    else:
                        nc.vector.tensor_copy(P_s[pr][:, ts(chh, 1024)], ps[:])
                # interleave+cast into the fp8 DoubleRow layout via gpsimd
                # casting DMAs (only engine allowed to cast in a DMA)
                Ppr = Pp_s[pr].rearrange("p (h i k) -> p h i k", h=2, i=2, k=S)
                for h in range(2):
                    for i2 in range(2):
                        nc.gpsimd.dma_start(
                            Ppr[:, h, i2, :],
                            P_s[pr][h * 64 + i2 * 32 : h * 64 + i2 * 32 + 32, :],
                        )

            for p in range(NPAIR):
                p_proj(p)
            if taps:
                nc.sync.dma_start(tap_d["dbg_P"][:], P_s[0][:])

            # transient pools
            e2_p = ctx.enter_context(tc.tile_pool(name="e2", bufs=3))
            rec_p = ctx.enter_context(tc.tile_pool(name="rec", bufs=2))
            rrs_p = ctx.enter_context(tc.tile_pool(name="rrs", bufs=2))
            fco_p = ctx.enter_context(tc.tile_pool(name="fco", bufs=2))

            # fc_out for one 128-row query block sb: 8 matmuls into one
            # 2-bank psum tile, one batched copy, one row-block DMA out.
            def fc_unit(sb, copy_eng="v"):
                ps = sc_ps.tile([128, 1024], f32, tag="sc", name=f"fc{sb}")
                for oc in range(2):
                    for p in range(NPAIR):
                        nc.tensor.matmul(
                            ps[:, ts(oc, 512)],
                            lhsT=ctxT[p][:, ts(sb, 128)],
                            rhs=ms_t[p][:, ts(oc, 512)],
                            start=(p == 0),
                            stop=(p == NPAIR - 1),
                        )
                fo = fco_p.tile([128, 1024], f32, tag="fco")
                if copy_eng == "v":
                    nc.vector.tensor_copy(fo[:], ps[:])
                else:
                    nc.scalar.copy(fo[:], ps[:])
                nc.sync.dma_start(out_d[ts(sb, 128), :], fo[:])

            # ---- attention main loop ----
            for cc in range(NCC):
                for pr in range(NPAIR):
                    # one 2-bank tile: [head A ctx | head B ctx], row 64 sumexp
                    AB = ctx_ps.tile([65, 1024], f32, tag="ctx", name=f"c{pr}_{cc}")
                    e2 = None
                    for jb in range(NJB):
                        jb2, i = jb // 2, jb % 2
                        if i == 0:
                            e2 = e2_p.tile([128, 2048], bf16, tag="e2")
                            e2_r = e2.rearrange("p (h i n) -> p h i n", h=2, i=2, n=512)
                        s_t = sc_ps.tile([128, 1024], f32, tag="sc")
                        Ppr = Pp_s[pr].rearrange("p (h i k) -> p h i k", h=2, i=2, k=S)
                        xqr = xq_s[pr].rearrange("p (h i k) -> p h i k", h=2, i=2, k=S)
                        for h in range(2):
                            nc.tensor.matmul(
                                s_t[:, ts(h, 512)],
                                lhsT=Ppr[:, h, :, ts(jb, 128)],
                                rhs=xqr[:, h, :, ts(cc, 512)],
                                start=True,
                                stop=True,
                                perf_mode=DROW,
                            )
                        dst = e2_r[:, :, i, :]
                        if jb in dve_jbs:
                            nc.vector.tensor_scalar(
                                out=dst.bitcast(i16),
                                in0=s_t[:],
                                scalar1=float(A16),
                                scalar2=float(B16),
                                op0=MUL,
                                op1=ADD,
                            )
                        else:
                            nc.scalar.activation(dst, s_t[:], EXP, scale=inv_sqrt_e)
                        if taps and pr == 0 and cc == 0 and jb == 1:
                            nc.sync.dma_start(tap_d["dbg_e2"][:], e2[:])
                        vpr = vp_t[jb2].rearrange(
                            "p (pr h i c) -> p pr h i c", pr=NPAIR, h=2, i=2, c=65
                        )
                        for h in range(2):
                            nc.tensor.matmul(
                                AB[:, ts(h, 512)],
                                lhsT=vpr[:, pr, h, i, :],
                                rhs=e2_r[:, h, i, :],
                                start=(jb == 0),
                                stop=(jb == NJB - 1),
                            )
                        # fc_out of the previous chunk rides in the gaps
                        if cc > 0 and jb == (4 if pr == 0 else 2):
                            fc_unit((cc - 1) * 4 + pr)

                    # ---- normalization for (pr, cc) ----
                    # approx recip over the whole tile (base partition 0);
                    # only row 64 (the sumexp row) is consumed downstream.
                    rec_c = rec_p.tile([65, 1024], f32, tag="rec")
                    nc.vector.reciprocal_approx_fast(rec_c[:], AB[:])
                    nc.sync.dma_start(rec_dram[pr][:, ts(cc, 512)], rec_c[64:65, :])
                    if taps and pr == 0 and cc == 0:
                        cu_tap = rec_p.tile([65, 1024], f32, tag="cutap")
                        nc.vector.tensor_copy(cu_tap[:], AB[:])
                        nc.sync.dma_start(tap_d["dbg_cu"][:], cu_tap[:, 0:512])
                    for hl in range(2):
                        rrs_c = rrs_p.tile([64, 512], f32, tag="rrs")
                        nc.sync.dma_start(
                            rrs_c[:],
                            rec_dram[pr][hl : hl + 1, ts(cc, 512)].partition_broadcast(64),
                        )
                        if hl == 0:
                            # partitions align: write ctxT rows 0-63 directly
                            nc.vector.tensor_mul(
                                ctxT[pr][0:64, ts(cc, 512)],
                                AB[0:64, ts(hl, 512)],
                                rrs_c[:],
                            )
                        else:
                            # rows 64-127 need a partition shift -> via DMA
                            tmp = rrs_p.tile([64, 512], f32, tag="tmp")
                            nc.vector.tensor_mul(tmp[:], AB[0:64, ts(hl, 512)], rrs_c[:])
                            nc.sync.dma_start(
                                ctxT[pr][64:128, ts(cc, 512)],
                                tmp[:].bitcast(f32r),
                            )
            if taps:
                for p, nm in enumerate(("dbg_ctxT", "dbg_ctxT1", "dbg_ctxT2", "dbg_ctxT3")):
                    nc.sync.dma_start(tap_d[nm][:], ctxT[p][:].bitcast(f32))
            for k, sb in enumerate(range((NCC - 1) * 4, NCC * 4)):
                fc_unit(sb, copy_eng="s" if k % 2 == 0 else "v")

    nc.compile()
    return nc


def make_group_weights(Wv, Wq, Wk, Wo):
    """Head-shared fused weights: G = Wq.T @ Wk; M_h = Wv.T @ Wo_h.T."""
    bf = ml_dtypes.bfloat16
    # scores = xq . (Wq.T Wk) . xk^T; the device computes P = lhsT.T @ xkT
    # with lhsT[d, c] needing (Wq.T Wk).T = Wk.T Wq
    G = (Wk.T @ Wq).astype(bf)
    gs = np.ascontiguousarray(np.concatenate([G, G], axis=0))  # (128, 64)
    M = np.empty((HEADS, HD, EMBED), np.float32)
    WvT = Wv.T.astype(np.float32)
    for h in range(HEADS):
        M[h] = WvT @ Wo[:, ts(h, HD)].T.astype(np.float32)
    return gs, M


def make_core_inputs(values, keys, queries, gs, M, n, g, S):
    """Host-side marshaling for core (n, g)."""
    bf = ml_dtypes.bfloat16
    cols = slice(g * GCOLS, (g + 1) * GCOLS)
    NPAIR = 4
    NJB2 = S // 256

    def xt(x):
        t = np.ascontiguousarray(x[n][:, cols].T.astype(bf))  # (512, S)
        return t.reshape(NPAIR, 128, S)

    # queries: fp8, DoubleRow-interleaved [pr][p][h, i2, n], d = h*64+i2*32+p
    f8 = ml_dtypes.float8_e4m3
    xqT = queries[n][:, cols].T  # (512, S) f32
    xqp = np.ascontiguousarray(
        xqT.reshape(NPAIR, 2, 2, 32, S).transpose(0, 3, 1, 2, 4).astype(f8)
    ).reshape(NPAIR, 32, 4 * S)

    xv = values[n][:, cols]  # (S, 512) f32
    vp = np.ones((NJB2, 128, NPAIR, 2, 2, 65), np.float32)
    # vp[jb2, p, pr, h, i, c<64] = xv[(2*jb2+i)*128 + p, (2pr+h)*64 + c]
    xvr = xv.reshape(NJB2, 2, 128, NPAIR, 2, HD)  # (jb2, i, p, pr, h, c)
    vp[..., :64] = xvr.transpose(0, 2, 3, 4, 1, 5)
    vp = np.ascontiguousarray(vp.astype(bf)).reshape(NJB2, 128, NPAIR * 260)

    ms = np.empty((NPAIR, 128, EMBED), np.float32)
    for pr in range(NPAIR):
        ms[pr, 0:64] = M[g * HPC + 2 * pr]
        ms[pr, 64:128] = M[g * HPC + 2 * pr + 1]

    return {
        "xk": xt(keys),
        "xq": xqp,
        "vp": vp,
        "gs": gs,
        "ms": ms,
    }


_PROG_CACHE = {}
TRACE = False
LAST_RESULTS = None


def kernel(values, keys, queries, mask, Wv, Wk, Wq, Wo, bo):
    global LAST_RESULTS
    from concourse.bass_utils import run_bass_kernel_spmd

    values = np.asarray(values, np.float32)
    keys = np.asarray(keys, np.float32)
    queries = np.asarray(queries, np.float32)
    Wv = np.asarray(Wv, np.float32)
    Wk = np.asarray(Wk, np.float32)
    Wq = np.asarray(Wq, np.float32)
    Wo = np.asarray(Wo, np.float32)
    bo = np.asarray(bo, np.float32)

    N, S, _ = queries.shape
    if S not in _PROG_CACHE:
        _PROG_CACHE[S] = build_program(S)
    nc = _PROG_CACHE[S]

    gs, M = make_group_weights(Wv, Wq, Wk, Wo)
    in_maps = [
        make_core_inputs(values, keys, queries, gs, M, c // 2, c % 2, S)
        for c in range(N_CORES)
    ]
    res = run_bass_kernel_spmd(
        nc, in_maps, core_ids=list(range(N_CORES)), trace=TRACE
    )
    LAST_RESULTS = res
    out = np.empty((N, S, EMBED), np.float32)
    for n in range(N):
        out[n] = res.results[2 * n]["out"] + res.results[2 * n + 1]["out"] + bo
    return out


# revision 36
# speedup vs baseline: 1.9534x; 1.9534x over previous
"""Trainium2 Bass kernel for 16-head self-attention (N=4, S=2048, E=1024).

Sharding: 8 cores = 4 batches x 2 head-groups (8 heads each).  Each core
computes its head-group's attention and a partial fc_out product
(S x 1024); the host sums the two partials per batch and adds the bias.

v2 structure (vs the v1 baseline):
  - Q/K projections fused into a single P = G @ Xk pass (G = Wq.T Wk,
    host-precomputed); scores = P.T @ Xq uses the raw query input.
  - V projection folded into fc_out: ctx uses raw Xv (host-marshaled
    with a fused ones column for sumexp), fc weights M_h = Wv.T Wo_h.T.
  - exp split between ScalarE (LUT exp) and DVE (Schraudolph bit-trick
    exp: i16 = round(x*a+b) bitcast bf16) so the Activation engine is no
    longer the serial wall.
  - fc_out software-pipelined one query-chunk behind attention so PE
    never stalls on the normalization chain; ACT does the fc PSUM->SBUF
    copies.
"""

import numpy as np
import ml_dtypes

EMBED = 1024
HEADS = 16
HD = 64
N_CORES = 8
HPC = 8  # heads per core
GCOLS = HPC * HD  # 512 embed columns per core

# jb indices (of 16 key-blocks) whose exp runs on DVE via Schraudolph.
DVE_JBS = (2, 6, 10, 13)
SCHRAUDOLPH_C = 0.0435


def ts(i, n):
    return slice(i * n, (i + 1) * n)


def build_program(S, dve_jbs=DVE_JBS, taps=False):
    import concourse.bass as bass
    import concourse.tile as tile
    import concourse.mybir as mybir
    from concourse import bacc

    f32 = mybir.dt.float32
    f32r = mybir.dt.float32r
    bf16 = mybir.dt.bfloat16
    i16 = mybir.dt.int16
    f8 = mybir.dt.float8e4
    EXP = mybir.ActivationFunctionType.Exp
    MUL = mybir.AluOpType.mult
    ADD = mybir.AluOpType.add
    DROW = mybir.MatmulPerfMode.DoubleRow

    NPAIR = 4
    NJB = S // 128  # 16 key blocks
    NJB2 = NJB // 2
    NCC = S // 512  # query chunks

    inv_sqrt_e = 1.0 / float(np.sqrt(EMBED))
    # Schraudolph constants: i16 = round(score * A16 + B16), bitcast bf16
    A16 = 128.0 * np.log2(np.e) * inv_sqrt_e
    B16 = 127.0 * 128.0 - SCHRAUDOLPH_C * 128.0

    nc = bacc.Bacc("TRN2", target_bir_lowering=False, debug=False)

    xk_d = nc.dram_tensor("xk", [NPAIR, 128, S], bf16, kind="ExternalInput").ap()
    xq_d = nc.dram_tensor("xq", [NPAIR, 128, S], bf16, kind="ExternalInput").ap()
    vp_d = nc.dram_tensor("vp", [NJB2, 128, NPAIR * 2 * 2 * 65], bf16, kind="ExternalInput").ap()
    gs_d = nc.dram_tensor("gs", [128, HD], bf16, kind="ExternalInput").ap()
    ms_d = nc.dram_tensor("ms", [NPAIR, 128, EMBED], f32r, kind="ExternalInput").ap()
    out_d = nc.dram_tensor("out", [S, EMBED], f32, kind="ExternalOutput").ap()
    rec_dram = nc.dram_tensor("rec_scratch", [NPAIR, 2, S], f32)
    tap_d = {}
    if taps:
        for nm, shape, dt_ in (
            ("dbg_P", [128, S], bf16),
            ("dbg_e2", [128, 2048], bf16),
            ("dbg_cu", [65, 512], f32),
            ("dbg_ctxT", [128, S], f32),
            ("dbg_ctxT1", [128, S], f32),
            ("dbg_ctxT2", [128, S], f32),
            ("dbg_ctxT3", [128, S], f32),
        ):
            tap_d[nm] = nc.dram_tensor(nm, shape, dt_, kind="ExternalOutput").ap()

    with tile.TileContext(nc) as tc:
        import contextlib

        with contextlib.ExitStack() as ctx:
            # ---- persistent pools ----
            const_p = ctx.enter_context(tc.tile_pool(name="const", bufs=1))
            pq_p = ctx.enter_context(tc.tile_pool(name="pq", bufs=1))
            vp_p = ctx.enter_context(tc.tile_pool(name="vp", bufs=1))
            ctxT_p = ctx.enter_context(tc.tile_pool(name="ctxT", bufs=1))
            wo_p = ctx.enter_context(tc.tile_pool(name="wo", bufs=1))
            # PSUM: sc slots 2 banks x2 bufs + ctx slots 2 banks x2 bufs = 8
            sc_ps = ctx.enter_context(tc.tile_pool(name="sc", bufs=2, space="PSUM"))
            ctx_ps = ctx.enter_context(tc.tile_pool(name="ctxps", bufs=2, space="PSUM"))

            gs_s = const_p.tile([128, HD], bf16, tag="gs")
            nc.sync.dma_start(gs_s[:], gs_d[:])
            ms_t = [wo_p.tile([128, EMBED], f32r, tag=f"ms{p}", name=f"ms{p}") for p in range(NPAIR)]

            P_s = [pq_p.tile([128, S], bf16, tag=f"P{p}", name=f"P{p}") for p in range(NPAIR)]
            xq_s = [pq_p.tile([128, S], bf16, tag=f"xq{p}", name=f"xq{p}") for p in range(NPAIR)]
            vp_t = [vp_p.tile([128, NPAIR * 260], bf16, tag=f"vp{j}", name=f"vp{j}") for j in range(NJB2)]
            ctxT = [ctxT_p.tile([128, S], f32r, tag=f"cx{p}", name=f"cx{p}") for p in range(NPAIR)]

            xk_t = [pq_p.tile([128, S], bf16, tag=f"xk{p}", name=f"xk{p}") for p in range(NPAIR)]
            for p in range(NPAIR):
                nc.sync.dma_start(xk_t[p][:], xk_d[p])
            nc.sync.dma_start(xq_s[0][:], xq_d[0])
            for j in range(NJB2):
                nc.sync.dma_start(vp_t[j][:], vp_d[j])
            for p in range(1, NPAIR):
                nc.sync.dma_start(xq_s[p][:], xq_d[p])
            for p in range(NPAIR):
                nc.sync.dma_start(ms_t[p][:], ms_d[p])

            # ---- P = G.T-stacked @ xk (key-side fused projection) ----
            # P(0) upfront; P(1..3) ride inside the cc=0 blocks.
            def p_proj(pr):
                for chh in range(S // 1024):
                    ps = sc_ps.tile([128, 1024], f32, tag="sc")
                    for ch2 in range(2):
                        for b in (0, 64):
                            nc.tensor.matmul(
                                ps[b : b + 64, ts(ch2, 512)],
                                lhsT=gs_s[b : b + 64, :],
                                rhs=xk_t[pr][b : b + 64, ts(chh * 2 + ch2, 512)],
                                start=True,
                                stop=True,
                            )
                    if chh % 2 == 0:
                        nc.scalar.copy(P_s[pr][:, ts(chh, 1024)], ps[:])
                    else:
                        nc.vector.tensor_copy(P_s[pr][:, ts(chh, 1024)], ps[:])

            for p in range(NPAIR):
                p_proj(p)
            if taps:
                nc.sync.dma_start(tap_d["dbg_P"][:], P_s[0][:])

            # transient pools
            e2_p = ctx.enter_context(tc.tile_pool(name="e2", bufs=3))
            rec_p = ctx.enter_context(tc.tile_pool(name="rec", bufs=2))
            rrs_p = ctx.enter_context(tc.tile_pool(name="rrs", bufs=2))
            fco_p = ctx.enter_context(tc.tile_pool(name="fco", bufs=2))

            # fc_out for one 128-row query block sb: 8 matmuls into one
            # 2-bank psum tile, one batched copy, one row-block DMA out.
            def fc_unit(sb, copy_eng="v"):
                ps = sc_ps.tile([128, 1024], f32, tag="sc", name=f"fc{sb}")
                for oc in range(2):
                    for p in range(NPAIR):
                        nc.tensor.matmul(
                            ps[:, ts(oc, 512)],
                            lhsT=ctxT[p][:, ts(sb, 128)],
                            rhs=ms_t[p][:, ts(oc, 512)],
                            start=(p == 0),
                            stop=(p == NPAIR - 1),
                        )
                fo = fco_p.tile([128, 1024], f32, tag="fco")
                if copy_eng == "v":
                    nc.vector.tensor_copy(fo[:], ps[:])
                else:
                    nc.scalar.copy(fo[:], ps[:])
                nc.sync.dma_start(out_d[ts(sb, 128), :], fo[:])

            # ---- attention main loop ----
            for cc in range(NCC):
                for pr in range(NPAIR):
                    # one 2-bank tile: [head A ctx | head B ctx], row 64 sumexp
                    AB = ctx_ps.tile([65, 1024], f32, tag="ctx", name=f"c{pr}_{cc}")
                    e2 = None
                    for jb in range(NJB):
                        jb2, i = jb // 2, jb % 2
                        if i == 0:
                            e2 = e2_p.tile([128, 2048], bf16, tag="e2")
                            e2_r = e2.rearrange("p (h i n) -> p h i n", h=2, i=2, n=512)
                        s_t = sc_ps.tile([128, 1024], f32, tag="sc")
                        for h in range(2):
                            nc.tensor.matmul(
                                s_t[:, ts(h, 512)],
                                lhsT=P_s[pr][h * 64 : h * 64 + 64, ts(jb, 128)],
                                rhs=xq_s[pr][h * 64 : h * 64 + 64, ts(cc, 512)],
                                start=True,
                                stop=True,
                            )
                        dst = e2_r[:, :, i, :]
                        if jb in dve_jbs:
                            nc.vector.tensor_scalar(
                                out=dst.bitcast(i16),
                                in0=s_t[:],
                                scalar1=float(A16),
                                scalar2=float(B16),
                                op0=MUL,
                                op1=ADD,
                            )
                        else:
                            nc.scalar.activation(dst, s_t[:], EXP, scale=inv_sqrt_e)
                        if taps and pr == 0 and cc == 0 and jb == 1:
                            nc.sync.dma_start(tap_d["dbg_e2"][:], e2[:])
                        vpr = vp_t[jb2].rearrange(
                            "p (pr h i c) -> p pr h i c", pr=NPAIR, h=2, i=2, c=65
                        )
                        for h in range(2):
                            nc.tensor.matmul(
                                AB[:, ts(h, 512)],
                                lhsT=vpr[:, pr, h, i, :],
                                rhs=e2_r[:, h, i, :],
                                start=(jb == 0),
                                stop=(jb == NJB - 1),
                            )
                        # fc_out of the previous chunk rides in the gaps
                        if cc > 0 and jb == (4 if pr == 0 else 2):
                            fc_unit((cc - 1) * 4 + pr)

                    # ---- normalization for (pr, cc) ----
                    # approx recip over the whole tile (base partition 0);
                    # only row 64 (the sumexp row) is consumed downstream.
                    rec_c = rec_p.tile([65, 1024], f32, tag="rec")
                    nc.vector.reciprocal_approx_fast(rec_c[:], AB[:])
                    nc.sync.dma_start(rec_dram[pr][:, ts(cc, 512)], rec_c[64:65, :])
                    if taps and pr == 0 and cc == 0:
                        cu_tap = rec_p.tile([65, 1024], f32, tag="cutap")
                        nc.vector.tensor_copy(cu_tap[:], AB[:])
                        nc.sync.dma_start(tap_d["dbg_cu"][:], cu_tap[:, 0:512])
                    for hl in range(2):
                        rrs_c = rrs_p.tile([64, 512], f32, tag="rrs")
                        nc.sync.dma_start(
                            rrs_c[:],
                            rec_dram[pr][hl : hl + 1, ts(cc, 512)].partition_broadcast(64),
                        )
                        if hl == 0:
                            # partitions align: write ctxT rows 0-63 directly
                            nc.vector.tensor_mul(
                                ctxT[pr][0:64, ts(cc, 512)],
                                AB[0:64, ts(hl, 512)],
                                rrs_c[:],
                            )
                        else:
                            # rows 64-127 need a partition shift -> via DMA
                            tmp = rrs_p.tile([64, 512], f32, tag="tmp")
                            nc.vector.tensor_mul(tmp[:], AB[0:64, ts(hl, 512)], rrs_c[:])
                            nc.sync.dma_start(
                                ctxT[pr][64:128, ts(cc, 512)],
                                tmp[:].bitcast(f32r),
                            )
            if taps:
                for p, nm in enumerate(("dbg_ctxT", "dbg_ctxT1", "dbg_ctxT2", "dbg_ctxT3")):
                    nc.sync.dma_start(tap_d[nm][:], ctxT[p][:].bitcast(f32))
            for k, sb in enumerate(range((NCC - 1) * 4, NCC * 4)):
                fc_unit(sb, copy_eng="s" if k % 2 == 0 else "v")

    nc.compile()
    return nc


def make_group_weights(Wv, Wq, Wk, Wo):
    """Head-shared fused weights: G = Wq.T @ Wk; M_h = Wv.T @ Wo_h.T."""
    bf = ml_dtypes.bfloat16
    # scores = xq . (Wq.T Wk) . xk^T; the device computes P = lhsT.T @ xkT
    # with lhsT[d, c] needing (Wq.T Wk).T = Wk.T Wq
    G = (Wk.T @ Wq).astype(bf)
    gs = np.ascontiguousarray(np.concatenate([G, G], axis=0))  # (128, 64)
    M = np.empty((HEADS, HD, EMBED), np.float32)
    WvT = Wv.T.astype(np.float32)
    for h in range(HEADS):
        M[h] = WvT @ Wo[:, ts(h, HD)].T.astype(np.float32)
    return gs, M


def make_core_inputs(values, keys, queries, gs, M, n, g, S):
    """Host-side marshaling for core (n, g)."""
    bf = ml_dtypes.bfloat16
    cols = slice(g * GCOLS, (g + 1) * GCOLS)
    NPAIR = 4
    NJB2 = S // 256

    def xt(x):
        t = np.ascontiguousarray(x[n][:, cols].T.astype(bf))  # (512, S)
        return t.reshape(NPAIR, 128, S)

    xv = values[n][:, cols]  # (S, 512) f32
    vp = np.ones((NJB2, 128, NPAIR, 2, 2, 65), np.float32)
    # vp[jb2, p, pr, h, i, c<64] = xv[(2*jb2+i)*128 + p, (2pr+h)*64 + c]
    xvr = xv.reshape(NJB2, 2, 128, NPAIR, 2, HD)  # (jb2, i, p, pr, h, c)
    vp[..., :64] = xvr.transpose(0, 2, 3, 4, 1, 5)
    vp = np.ascontiguousarray(vp.astype(bf)).reshape(NJB2, 128, NPAIR * 260)

    ms = np.empty((NPAIR, 128, EMBED), np.float32)
    for pr in range(NPAIR):
        ms[pr, 0:64] = M[g * HPC + 2 * pr]
        ms[pr, 64:128] = M[g * HPC + 2 * pr + 1]

    return {
        "xk": xt(keys),
        "xq": xt(queries),
        "vp": vp,
        "gs": gs,
        "ms": ms,
    }


_PROG_CACHE = {}
TRACE = False
LAST_RESULTS = None


def kernel(values, keys, queries, mask, Wv, Wk, Wq, Wo, bo):
    global LAST_RESULTS
    from concourse.bass_utils import run_bass_kernel_spmd

    values = np.asarray(values, np.float32)
    keys = np.asarray(keys, np.float32)
    queries = np.asarray(queries, np.float32)
    Wv = np.asarray(Wv, np.float32)
    Wk = np.asarray(Wk, np.float32)
    Wq = np.asarray(Wq, np.float32)
    Wo = np.asarray(Wo, np.float32)
    bo = np.asarray(bo, np.float32)

    N, S, _ = queries.shape
    if S not in _PROG_CACHE:
        _PROG_CACHE[S] = build_program(S)
    nc = _PROG_CACHE[S]

    gs, M = make_group_weights(Wv, Wq, Wk, Wo)
    in_maps = [
        make_core_inputs(values, keys, queries, gs, M, c // 2, c % 2, S)
        for c in range(N_CORES)
    ]
    res = run_bass_kernel_spmd(
        nc, in_maps, core_ids=list(range(N_CORES)), trace=TRACE
    )
    LAST_RESULTS = res
    out = np.empty((N, S, EMBED), np.float32)
    for n in range(N):
        out[n] = res.results[2 * n]["out"] + res.results[2 * n + 1]["out"] + bo
    return out
